# revision 4
# baseline (speedup 1.0000x reference)
"""DeepSQN (spiking CNN, T=8) forward pass on 8 Trainium2 NeuronCores — v2.

Sharding: data-parallel over batch B=128 -> 16 samples/core; each BN layer
AllReduces tiny per-partition (sum, sumsq) stats [128,2] fp32 across cores.

Key optimizations vs the v1 baseline (167.7us -> 101.9us cost-model):
- all matmul operands fp16: same PE rate as bf16 with ~8x better mantissa,
  so conv1 runs a single pass (no hi/lo operand splitting)
- conv1 contracts K=128 in 2 accumulation steps by folding the kernel-column
  block offset into the partition dim via a column-shifted input duplicate
  (host prep): 12.8k PE columns instead of 51.2k
- conv3 contracts K=128 in 6 steps (was K=64 in 9): kx in {0,1} folded into
  partitions via a j-shifted duplicate of the spike map, rebuilt per timestep
  with 4 small DMAs (3 HWDGE + 1 SWDGE queue); kx=2 runs at K=64
- fc1 flipped to M=hid-chunk(128) x N=(t,n): 25.1k columns instead of 50.2k,
  sliced per timestep-pair and pipelined with the LIF3 membrane recursion;
  LIF4 + fco ride the same pipeline. Each accumulation group owns a
  PSUM-bank-base tile (sub-bank column windows fault on HW)
- LIF v-states double-buffered so the spike compare is off the recursion
  critical path; first-step spikes in threshold form (skip the charge wait)
- weights ship in 3 packed DMAs + fc1 weights in 17 chunks pinned behind
  phase-B progress (big transfers otherwise block the BN stats hops on the
  shared DMA engines); input x in 8 chunks so conv1 starts after ~1/8 of it
- junk warm-up matmuls at t0 and Pool-paced junk matmuls through each BN
  stats round-trip keep the PE p-state/HAM busy-stretch clock running, so
  every phase starts at the full 2.4 GHz rate
- BN affine chains collapsed: combine matrices pre-scaled by 1/count, one
  ACT-Sqrt per layer, thresholds via single tensor_scalar ops
"""
import os
import numpy as np
import ml_dtypes

import concourse.bass as bass
import concourse.mybir as mybir
import concourse.tile as tile
from concourse import bacc
from concourse.bass_utils import run_bass_kernel_spmd
from contextlib import ExitStack

F32 = mybir.dt.float32
F16 = mybir.dt.float16
AF = mybir.ActivationFunctionType
OP = mybir.AluOpType

N_CORES = 8
T = 8
B_LOC = 16
EPS = 1e-5

CNT1 = 128 * 400          # BN1: T collapses (replicated input), count = B*20*20
CNT2 = T * 128 * 81
CNT3 = T * 128 * 49

CK = [1.0 / (1.0 - 0.5 ** k) for k in range(1, 9)]
# per-partition sum over t of y_t in terms of sum(C_k)
WSUM = [4.0, 2.0, 0.0, 1.0, 0.0, 0.0, 0.0, 1.0]
# y_t composition for t>=2 (0-based): base ('c' = C_k index, 'y' = y_t index)
YBASE = {2: ("c", 0), 3: ("y", 1), 4: ("c", 0), 5: ("c", 2), 6: ("c", 0), 7: ("y", 3)}

DEBUG = bool(int(os.environ.get("KERNEL_DEBUG", "0")))
# fc1 time-granularity: 2/4/8 timesteps per accumulation slice
FC1_TG = int(os.environ.get("KERNEL_FC1_TG", "2"))
NO_FC1 = bool(int(os.environ.get("KERNEL_NO_FC1", "0")))
NO_LIF4 = bool(int(os.environ.get("KERNEL_NO_LIF4", "0")))
# Replace collectives with local DMA copies and build for 1 core — used only
# for cost-model timing (TimelineSim); numerics are wrong in this mode.
NO_CC = bool(int(os.environ.get("KERNEL_NO_CC", "0")))
# With NO_CC: scale BN counts to the local shard so a single-core sim
# reproduces "BN over 16 samples" exactly (debug aid only).
LOCAL_STATS = bool(int(os.environ.get("KERNEL_LOCAL_STATS", "0")))

_CACHE = {}


def _f16(x):
    return np.asarray(x, np.float32).astype(np.float16)


def _prep_shared(inp):
    w1 = np.asarray(inp["conv1_w"], np.float32)
    w2 = np.asarray(inp["conv2_w"], np.float32)
    w3 = np.asarray(inp["conv3_w"], np.float32)
    wf = np.asarray(inp["fc1_w"], np.float32)
    wo = np.asarray(inp["fco_w"], np.float32)

    # conv1 lhsT [(b,c,ry,rx)=128, a=2, oc=32]: w1[oc, c, 4a+ry, 4b+rx]
    w1p = np.zeros((2, 4, 4, 4, 2, 32), np.float32)      # b,c,ry,rx,a,oc
    for b in range(2):
        for a in range(2):
            w1p[b, :, :, :, a, :] = w1.transpose(1, 2, 3, 0)[:, 4 * a:4 * a + 4,
                                                             4 * b:4 * b + 4, :]
    w1p = w1p.reshape(128, 2, 32)

    # conv2 lhsT [(dy,dx,c)=128, (A,B)=4, oc=64]
    w2b = w2.reshape(64, 32, 2, 2, 2, 2)                 # oc,c,A,dy,B,dx
    w2r = np.ascontiguousarray(w2b.transpose(3, 5, 1, 2, 4, 0)).reshape(128, 4, 64)

    # conv3 lhsT: kx in {0,1} folded into K via partition (d,c2); kx=2 alone
    w3t = w3.transpose(1, 2, 3, 0)                       # c2, ky, kx, oc
    w3a = np.concatenate([
        np.ascontiguousarray(w3t[:, :, 0, :]).reshape(64, 192),
        np.ascontiguousarray(w3t[:, :, 1, :]).reshape(64, 192)], axis=0)
    w3b = np.concatenate([
        np.ascontiguousarray(w3t[:, :, 2, :]).reshape(64, 192),
        np.zeros((64, 192), np.float32)], axis=0)        # upper half unused

    # fco lhsT [hid_low=128, hc=4, k=2]  (hid = hc*128 + low)
    worr = np.ascontiguousarray(wo.reshape(2, 4, 128).transpose(2, 1, 0))

    # small fp16 pack: [128, 256 + 192 + 192 + 8] = [128, 648]; w1 ships alone
    wsmall = np.concatenate([
        w2r.reshape(128, 256), w3a, w3b, worr.reshape(128, 8)], axis=1)

    # fc1 rhs-weights [c=64 dup to 128, ij=49, hc=4, h=128]
    # wf[h_global, c*49 + i*7 + j]; h_global = hc*128 + h
    wfr = wf.reshape(4, 128, 64, 49)                     # hc, h, c, ij
    wfp = np.ascontiguousarray(wfr.transpose(2, 3, 0, 1)).reshape(64, 49 * 4 * 128)
    wfd = np.concatenate([wfp, wfp], axis=0)             # [128, 25088]

    div = N_CORES if LOCAL_STATS else 1
    p = np.arange(128)
    cmb1 = (p[:, None] % 32 == p[None, :] % 32).astype(np.float32) / (CNT1 // div)
    cmb2 = (p[:, None] % 64 == p[None, :] % 64).astype(np.float32)

    g1 = np.tile(np.asarray(inp["bn1_g"], np.float32), 4)
    b1 = np.tile(np.asarray(inp["bn1_b"], np.float32), 4)
    # BN1 thresholds: thr_k = ckt_pre_k * sd + m, ckt_pre_k = (c_k - b)/g
    ckt_pre = (np.asarray(CK, np.float32)[None, :] - b1[:, None]) / g1[:, None]

    vecs = np.zeros((128, 12), np.float32)
    vecs[:, 0] = 0.5 * np.tile(np.asarray(inp["bn2_g"], np.float32), 2)   # ghalf2
    vecs[:, 1] = -vecs[:, 0]                                              # ghalfneg2
    vecs[:, 2] = 0.5 * np.tile(np.asarray(inp["bn2_b"], np.float32), 2)   # bhalf2
    vecs[:, 3] = 0.5 * np.tile(np.asarray(inp["bn3_g"], np.float32), 2)
    vecs[:, 4] = -vecs[:, 3]
    vecs[:, 5] = 0.5 * np.tile(np.asarray(inp["bn3_b"], np.float32), 2)
    vecs[:, 6:10] = 0.5 * np.asarray(inp["fc1_b"], np.float32).reshape(4, 128).T
    vecs[0:2, 10] = np.asarray(inp["fco_b"], np.float32)

    wpack32 = np.concatenate([cmb1, cmb2 / (CNT2 // div), cmb2 / (CNT3 // div),
                              vecs, ckt_pre], axis=1)     # [128, 128*3+12+8]

    return {"w1pack": _f16(w1p.reshape(128, 64)), "wsmall": _f16(wsmall),
            "wfd": _f16(wfd),
            "wpack32": np.ascontiguousarray(wpack32, np.float32)}


def _prep_core(x_shard):
    # xprep [128=(b,c,ry,rx), (n,P,C)=16*21*21]: x[n, c, 4P+ry, 4C+4b+rx]
    x = np.asarray(x_shard, np.float32)
    xb = x.reshape(B_LOC, 4, 21, 4, 21, 4)               # n,c,P,ry,C,rx
    xprep = np.zeros((2, 4, 4, 4, B_LOC, 21, 21), np.float32)  # b,c,ry,rx,n,P,C
    xt = xb.transpose(1, 3, 5, 0, 2, 4)                  # c,ry,rx,n,P,C
    xprep[0] = xt
    xprep[1, :, :, :, :, :, 0:20] = xt[:, :, :, :, :, 1:21]
    return {"xprep": _f16(xprep.reshape(128, B_LOC * 441))}


def build_nc():
    nc = bacc.Bacc("TRN2", target_bir_lowering=False, debug=False,
                   num_devices=1 if NO_CC else N_CORES)

    dram_in = {
        "xprep": nc.dram_tensor("xprep", [128, B_LOC * 441], F16,
                                kind="ExternalInput"),
        "w1pack": nc.dram_tensor("w1pack", [128, 64], F16, kind="ExternalInput"),
        "wsmall": nc.dram_tensor("wsmall", [128, 648], F16, kind="ExternalInput"),
        "wfd": nc.dram_tensor("wfd", [128, 25088], F16, kind="ExternalInput"),
        "wpack32": nc.dram_tensor("wpack32", [128, 404], F32,
                                  kind="ExternalInput"),
    }
    out_d = nc.dram_tensor("out", [2, B_LOC], F32, kind="ExternalOutput")
    dbg = {}
    if DEBUG:
        for nm, sh, dt in [("d_y1", [128, 1600], F16), ("d_thr", [128, 8], F32),
                           ("d_c1", [128, 648], F16), ("d_s2", [128, T, 648], F16),
                           ("d_y31", [128, 392], F16), ("d_s3", [128, T, 392], F16),
                           ("d_xh4", [128, 8, 4, 8], F32), ("d_st1", [128, 2], F32)]:
            dbg[nm] = nc.dram_tensor(nm, sh, dt, kind="ExternalOutput")

    _emit(nc, dram_in, out_d, dbg)
    nc.compile()
    return nc


def _emit(nc, dram_in, out_d, dbg):
    STAGE = int(os.environ.get("KERNEL_STAGE", "99"))
    with tile.TileContext(nc) as tc, ExitStack() as ctx:
        per = ctx.enter_context(tc.tile_pool(name="persist", bufs=1))
        dram = ctx.enter_context(tc.tile_pool(name="drampool", bufs=1, space="DRAM"))
        psum_s = ctx.enter_context(tc.tile_pool(name="psum_s", bufs=1, space="PSUM"))

        w1pack = per.tile([128, 64], F16)
        nc.sync.dma_start(out=w1pack, in_=dram_in["w1pack"].ap())
        w1p = w1pack.rearrange("p (a o) -> p a o", a=2)

        wsmall = per.tile([128, 648], F16)
        w2r = wsmall[:, 0:256].rearrange("p (a o) -> p a o", a=4)
        w3a = wsmall[:, 256:448].rearrange("p (a o) -> p a o", a=3)
        w3b = wsmall[:, 448:640].rearrange("p (a o) -> p a o", a=3)
        wor = wsmall[:, 640:648].rearrange("p (a o) -> p a o", a=4)

        # --- PE p-state keep-warm: the cost model (and real HAM) throttles the
        # PE after idle gaps; junk matmuls at t0 and paced through the BN stats
        # round-trips keep the busy-stretch clock running so real phases start
        # at full rate. Paced via the otherwise-idle Pool engine.
        jnk = per.tile([128, 512], F16)
        nc.gpsimd.memset(jnk, 1.0)
        jpace = per.tile([128, 1536], F16)
        nc.gpsimd.memset(jpace, 0.0)
        psw = psum_s.tile([128, 512], F32, name="psw")

        def pe_warm(n, N=64):
            for _ in range(n):
                nc.tensor.matmul(psw[:, 0:N], jnk[:, 0:128], jnk[:, 0:N],
                                 start=True, stop=True)

        def pe_pace(dep_col, links):
            # Pool-engine pacing chain (affine_select is Pool-legal); each
            # link ~1.5us, with a junk matmul chained off it to keep the PE
            # busy-stretch clock alive through the BN round-trip.
            nc.gpsimd.affine_select(
                out=jpace[:, 0:1], in_=dep_col, compare_op=OP.is_ge,
                fill=0.0, base=0, pattern=[[0, 1]], channel_multiplier=0)
            for _ in range(links):
                nc.gpsimd.affine_select(
                    out=jpace[:, 0:1024], in_=jpace[:, 0:1024],
                    compare_op=OP.is_ge, fill=0.0, base=0,
                    pattern=[[0, 1024]], channel_multiplier=0)
                nc.tensor.matmul(psw[:, 0:128], jnk[:, 0:128],
                                 jpace[:, 0:128], start=True, stop=True)

        pe_warm(24)

        wpack32 = per.tile([128, 404], F32)
        cmb1 = wpack32[:, 0:128]
        cmb2s2 = wpack32[:, 128:256]
        cmb2s3 = wpack32[:, 256:384]
        vecs = wpack32[:, 384:396]
        ckt_pre = wpack32[:, 396:404]

        def stats_trip(sum_ap, sq_ap, name, pre_links=4, post_links=1):
            pe_pace(sum_ap, pre_links)
            s_loc = per.tile([128, 2], F32, name=f"sloc_{name}")
            nc.vector.tensor_copy(s_loc[:, 0:1], sum_ap)
            nc.vector.tensor_copy(s_loc[:, 1:2], sq_ap)
            arin = dram.tile([128, 2], F32, name=f"ari_{name}")
            arout = dram.tile([128, 2], F32, name=f"aro_{name}")
            nc.sync.dma_start(out=arin, in_=s_loc)
            if NO_CC:
                nc.sync.dma_start(out=arout, in_=arin)
            else:
                nc.gpsimd.collective_compute(
                    "AllReduce", OP.add, replica_groups=[list(range(N_CORES))],
                    ins=[arin.opt()], outs=[arout.opt()])
            s_glob = per.tile([128, 2], F32, name=f"sg_{name}")
            nc.sync.dma_start(out=s_glob, in_=arout)
            # combine channel-duplicated partitions; cmb pre-scaled by 1/cnt
            cmb = {"bn1": cmb1, "bn2": cmb2s2, "bn3": cmb2s3}[name]
            pb = psum_s.tile([128, 2], F32, tag="pb")
            nc.tensor.matmul(pb, cmb, s_glob, start=True, stop=True)
            mv = per.tile([128, 2], F32, name=f"mv_{name}")   # [m, E[y^2]]
            nc.vector.tensor_copy(mv, pb)
            pe_pace(mv[:, 0:1], post_links)
            # var = E[y^2] - m^2 ; sd = sqrt(var + EPS)
            msq = per.tile([128, 1], F32, name=f"msq_{name}")
            nc.vector.tensor_scalar(msq, mv[:, 0:1], mv[:, 0:1], None, op0=OP.mult)
            var = per.tile([128, 1], F32, name=f"var_{name}")
            nc.vector.tensor_tensor(var, mv[:, 1:2], msq, op=OP.subtract)
            sd = per.tile([128, 1], F32, name=f"sd_{name}")
            eps_b = per.tile([128, 1], F32, name=f"eps_{name}")
            nc.vector.memset(eps_b, EPS)
            nc.scalar.activation(sd, var, AF.Sqrt, bias=eps_b[:, :])
            return mv, sd

        # ================= conv1 =================
        y1 = per.tile([128, 1600], F16)
        acc1 = per.tile([128, 8], F32)
        acq1 = per.tile([128, 8], F32)
        with tc.tile_pool(name="xin", bufs=1) as xin, \
             tc.tile_pool(name="sq1p", bufs=2) as sq1p, \
             tc.tile_pool(name="ps1", bufs=2, space="PSUM") as ps1p:
            xprep = xin.tile([128, B_LOC * 441], F16)
            CH = 2 * 441
            for nch in range(8):
                nc.sync.dma_start(out=xprep[:, nch * CH:(nch + 1) * CH],
                                  in_=dram_in["xprep"].ap()[:, nch * CH:(nch + 1) * CH])
            nc.sync.dma_start(out=wsmall, in_=dram_in["wsmall"].ap())
            nc.sync.dma_start(out=wpack32, in_=dram_in["wpack32"].ap())
            xp4 = xprep.rearrange("k (n P C) -> k n P C", n=B_LOC, P=21)
            for nchunk in range(8):
                n0 = nchunk * 2
                ps = ps1p.tile([128, 200], F32, tag="c1ps", bufs=2)
                for par in range(4):
                    dy, dx = par // 2, par % 2
                    for a in range(2):
                        rhs = xp4[:, n0:n0 + 2,
                                  dy + a: dy + a + 19: 2,
                                  dx: dx + 19: 2]
                        nc.tensor.matmul(
                            ps[par * 32:(par + 1) * 32, :],
                            w1p[:, a, :], rhs,
                            start=(a == 0), stop=(a == 1),
                            tile_position=(0, 32 * par))
                ysl = y1[:, nchunk * 200:(nchunk + 1) * 200]
                nc.scalar.activation(ysl, ps, AF.Copy,
                                     accum_out=acc1[:, nchunk:nchunk + 1])
                sq = sq1p.tile([128, 200], F16, name="sq1", tag="sq1", bufs=2)
                nc.vector.scalar_tensor_tensor(
                    sq, ysl, 1.0, ysl, op0=OP.bypass, op1=OP.mult,
                    accum_out=acq1[:, nchunk:nchunk + 1])

        sum1 = per.tile([128, 1], F32)
        nc.vector.tensor_reduce(sum1, acc1, axis=mybir.AxisListType.X, op=OP.add)
        y1sq = per.tile([128, 1], F32)
        nc.vector.tensor_reduce(y1sq, acq1, axis=mybir.AxisListType.X, op=OP.add)

        if STAGE < 1:
            nc.sync.dma_start(out=out_d.ap(), in_=acc1[0:8, :])
            return

        # ================= BN1 + thresholds =================
        mv1, sd1 = stats_trip(sum1, y1sq, "bn1")
        thr = per.tile([128, 8], F32)
        nc.vector.tensor_scalar(thr, ckt_pre, sd1[:, :], mv1[:, 0:1],
                                op0=OP.mult, op1=OP.add)

        if DEBUG:
            nc.sync.dma_start(out=dbg["d_y1"].ap(), in_=y1)
            nc.sync.dma_start(out=dbg["d_thr"].ap(), in_=thr)
            nc.sync.dma_start(out=dbg["d_st1"].ap(), in_=mv1)

        if STAGE < 2:
            nc.sync.dma_start(out=out_d.ap(), in_=thr[0:4, 0:8])
            return

        # ================= g-maps + conv2 =================
        wf_sb = per.tile([128, 25088], F16)

        acc2 = per.tile([128, 8], F32)
        acq2 = per.tile([128, 8], F32)
        cy = per.tile([128, 14, 648], F16)    # C_1..C_8 then y_2..y_7
        with tc.tile_pool(name="gmaps", bufs=3) as gp, \
             tc.tile_pool(name="sq2p", bufs=2) as sq2p, \
             tc.tile_pool(name="ps2", bufs=2, space="PSUM") as ps2p:
            for k in range(8):
                g = gp.tile([128, 1600], F16, name=f"g{k}", tag="g", bufs=3)
                nc.vector.tensor_scalar(g, y1, thr[:, k:k + 1], None, op0=OP.is_ge)
                ps = ps2p.tile([128, 2, 512], F32, tag="c2ps", bufs=2)
                g4 = g.rearrange("p (n i j) -> p n i j", n=B_LOC, i=10)
                for gh in range(2):
                    for nch in range(2):
                        n0 = gh * 8 + nch * 4
                        for ab in range(4):
                            A, Bo = ab // 2, ab % 2
                            rhs = g4[:, n0:n0 + 4, A:A + 9, Bo:Bo + 9]
                            nc.tensor.matmul(
                                ps[gh * 64:(gh + 1) * 64, nch, 0:324],
                                w2r[:, ab, :], rhs,
                                start=(ab == 0), stop=(ab == 3),
                                tile_position=(0, 64 * gh))
                nc.scalar.activation(
                    cy[:, k, :].rearrange("p (a b) -> p a b", a=2), ps[:, :, 0:324],
                    AF.Copy, accum_out=acc2[:, k:k + 1])

                # interleave y_t composition + squares (y_0=C_1, y_1=C_2)
                if k >= 2:
                    kind, bi = YBASE[k]
                    # "y" base with bi<2 aliases C_{bi+1}
                    base = (cy[:, bi, :] if (kind == "c" or bi < 2)
                            else cy[:, 6 + bi, :])
                    yt = cy[:, 6 + k, :]
                    nc.vector.tensor_tensor(yt, cy[:, k, :], cy[:, k - 1, :],
                                            op=OP.subtract)
                    nc.vector.tensor_tensor(yt, yt, base, op=OP.add)
                ysrc = cy[:, k, :] if k < 2 else cy[:, 6 + k, :]
                sq = sq2p.tile([128, 648], F16, name="sq2", tag="sq2", bufs=2)
                nc.scalar.activation(sq, ysrc, AF.Square,
                                     accum_out=acq2[:, k:k + 1])
                # prefetch fc1 weights through phase B's idle DMA window;
                # the 1-elem copy pins each chunk behind this k's g-map so the
                # scheduler can't hoist the transfer into the BN1 round-trip
                if 1 <= k < 7:
                    for cc in range(3 if k < 6 else 2):
                        c17 = (k - 1) * 3 + cc
                        lo = c17 * 1480
                        hi = min(lo + 1480, 25088)
                        if lo >= 25088:
                            continue
                        nc.vector.tensor_copy(wf_sb[0:1, lo:lo + 1], g[0:1, 0:1])
                        nc.sync.dma_start(out=wf_sb[:, lo:hi],
                                          in_=dram_in["wfd"].ap()[:, lo:hi])

        if DEBUG:
            nc.sync.dma_start(out=dbg["d_c1"].ap(), in_=cy[:, 0, :])

        sum2 = per.tile([128, 1], F32)
        nc.vector.memset(sum2, 0.0)
        for k in range(8):
            if WSUM[k] != 0.0:
                nc.vector.scalar_tensor_tensor(
                    sum2, acc2[:, k:k + 1], WSUM[k], sum2, op0=OP.mult, op1=OP.add)
        sq2r = per.tile([128, 1], F32)
        nc.vector.tensor_reduce(sq2r, acq2, axis=mybir.AxisListType.X, op=OP.add)

        if STAGE < 3:
            nc.sync.dma_start(out=out_d.ap(), in_=acc2[0:4, :])
            return

        # ================= BN2 + LIF2 + conv3 =================
        mv2, sd2 = stats_trip(sum2, sq2r, "bn2")
        r2 = per.tile([128, 1], F32)
        nc.vector.reciprocal(r2, sd2)
        ha2 = per.tile([128, 1], F32)
        nc.vector.tensor_scalar(ha2, vecs[:, 0:1], r2[:, :], None, op0=OP.mult)
        han2 = per.tile([128, 1], F32)
        nc.vector.tensor_scalar(han2, vecs[:, 1:2], r2[:, :], None, op0=OP.mult)
        hc2 = per.tile([128, 1], F32)
        nc.vector.tensor_scalar(hc2, mv2[:, 0:1], han2[:, :], vecs[:, 2:3],
                                op0=OP.mult, op1=OP.add)

        # first-step spikes in threshold form: s_0 = [y_0 >= (1-hc)/ha]
        thr2 = per.tile([128, 1], F32)
        nc.vector.tensor_scalar(thr2, hc2, -1.0, 1.0, op0=OP.mult, op1=OP.add)
        ra2 = per.tile([128, 1], F32)
        nc.vector.reciprocal(ra2, ha2)
        nc.vector.tensor_scalar(thr2, thr2, ra2[:, :], None, op0=OP.mult)

        s2_all = per.tile([128, T, 648], F16)
        acc3 = per.tile([128, 8], F32)
        acq3 = per.tile([128, 8], F32)
        y3_all = per.tile([128, T, 392], F16)
        xh2_all = per.tile([128, T, 648], F32)
        lif2_v = [per.tile([128, 648], F32, name="lif2_va"),
                  per.tile([128, 648], F32, name="lif2_vb")]
        with tc.tile_pool(name="lif2p", bufs=2) as l2p, \
             tc.tile_pool(name="sq3p", bufs=2) as sq3p, \
             tc.tile_pool(name="s2dp", bufs=3) as s2dp, \
             tc.tile_pool(name="ps3", bufs=3, space="PSUM") as ps3p:
            for t in range(8):
                ysrc = cy[:, t, :] if t < 2 else cy[:, 6 + t, :]
                vcur, vprev = lif2_v[t % 2], lif2_v[1 - t % 2]
                if t == 0:
                    nc.vector.tensor_scalar(
                        s2_all[:, 0, :], ysrc, thr2[:, :], None, op0=OP.is_ge)
                    nc.scalar.activation(vcur, ysrc, AF.Identity,
                                         bias=hc2[:, :], scale=ha2[:, :])
                else:
                    nc.scalar.activation(xh2_all[:, t, :], ysrc, AF.Identity,
                                         bias=hc2[:, :], scale=ha2[:, :])
                    u = l2p.tile([128, 648], F32, name=f"u2_{t}", tag="u", bufs=2)
                    nc.vector.scalar_tensor_tensor(
                        u, vprev, 1.0, vprev, op0=OP.is_lt, op1=OP.mult)
                    nc.vector.scalar_tensor_tensor(
                        vcur, u, 0.5, xh2_all[:, t, :], op0=OP.mult, op1=OP.add)
                    nc.vector.tensor_scalar(
                        s2_all[:, t, :], vcur, 1.0, None, op0=OP.is_ge)

                # conv3 for this t: build [(d,c2), gh, (n,i,j)] with d = kx
                # shift via 4 DMAs (2 HWDGE + 2 SWDGE), then 6 accumulation
                # steps per gh: ky x {kx01 at K=128, kx=2 at K=64}
                s2d = s2dp.tile([128, 2, 648], F16, tag="s2d", bufs=3)
                nc.sync.dma_start(out=s2d[0:64, 0, :], in_=s2_all[0:64, t, :])
                nc.sync.dma_start(out=s2d[0:64, 1, :], in_=s2_all[64:128, t, :])
                nc.sync.dma_start(out=s2d[64:128, 0, 0:647],
                                  in_=s2_all[0:64, t, 1:648])
                nc.gpsimd.dma_start(out=s2d[64:128, 1, 0:647],
                                    in_=s2_all[64:128, t, 1:648])
                ps = ps3p.tile([128, 392], F32, tag="c3ps", bufs=3)
                s2v = s2d.rearrange("p g (n i j) -> p g n i j", n=8, i=9)
                for gh in range(2):
                    for ky in range(3):
                        rhs = s2v[:, gh, :, ky:ky + 7, 0:7]
                        nc.tensor.matmul(
                            ps[gh * 64:(gh + 1) * 64, :],
                            w3a[:, ky, :], rhs,
                            start=(ky == 0), stop=False,
                            tile_position=(0, 64 * gh))
                        rhsb = s2v[0:64, gh, :, ky:ky + 7, 2:9]
                        nc.tensor.matmul(
                            ps[gh * 64:(gh + 1) * 64, :],
                            w3b[0:64, ky, :], rhsb,
                            start=False, stop=(ky == 2),
                            tile_position=(0, 64 * gh))
                nc.scalar.activation(y3_all[:, t, :], ps, AF.Copy,
                                     accum_out=acc3[:, t:t + 1])
                sq = sq3p.tile([128, 392], F16, name="sq3", tag="sq3", bufs=2)
                nc.scalar.activation(sq, y3_all[:, t, :], AF.Square,
                                     accum_out=acq3[:, t:t + 1])

        if DEBUG:
            nc.sync.dma_start(out=dbg["d_s2"].ap(), in_=s2_all)
            nc.sync.dma_start(out=dbg["d_y31"].ap(), in_=y3_all[:, 0, :])

        sum3 = per.tile([128, 1], F32)
        nc.vector.tensor_reduce(sum3, acc3, axis=mybir.AxisListType.X, op=OP.add)
        sq3r = per.tile([128, 1], F32)
        nc.vector.tensor_reduce(sq3r, acq3, axis=mybir.AxisListType.X, op=OP.add)

        if STAGE < 4:
            nc.sync.dma_start(out=out_d.ap(), in_=acc3[0:4, :])
            return

        # ================= BN3 + LIF3 + fc1 + LIF4 + fco =================
        mv3, sd3 = stats_trip(sum3, sq3r, "bn3", post_links=2)
        r3 = per.tile([128, 1], F32)
        nc.vector.reciprocal(r3, sd3)
        ha3 = per.tile([128, 1], F32)
        nc.vector.tensor_scalar(ha3, vecs[:, 3:4], r3[:, :], None, op0=OP.mult)
        han3 = per.tile([128, 1], F32)
        nc.vector.tensor_scalar(han3, vecs[:, 4:5], r3[:, :], None, op0=OP.mult)
        hc3 = per.tile([128, 1], F32)
        nc.vector.tensor_scalar(hc3, mv3[:, 0:1], han3[:, :], vecs[:, 5:6],
                                op0=OP.mult, op1=OP.add)

        thr3 = per.tile([128, 1], F32)
        nc.vector.tensor_scalar(thr3, hc3, -1.0, 1.0, op0=OP.mult, op1=OP.add)
        ra3 = per.tile([128, 1], F32)
        nc.vector.reciprocal(ra3, ha3)
        nc.vector.tensor_scalar(thr3, thr3, ra3[:, :], None, op0=OP.mult)

        wf4 = wf_sb.rearrange("p (ij hc h) -> p ij hc h", ij=49, hc=4)
        s3_all = per.tile([128, T, 392], F16)
        lif3_v = [per.tile([128, 392], F32, name="lif3_va"),
                  per.tile([128, 392], F32, name="lif3_vb")]
        out_t = per.tile([2, B_LOC], F32)
        with tc.tile_pool(name="lif3p", bufs=2) as l3p, \
             tc.tile_pool(name="psf", bufs=4, space="PSUM") as psfp, \
             tc.tile_pool(name="pso", bufs=2, space="PSUM") as psop:
            # each matmul accumulation group must target a PSUM-bank base:
            # sub-bank column windows of a shared tile fault on HW, so psF/psO
            # tiles rotate through bank-base pool slots per group
            osum = per.tile([2, 16], F32)
            xh4 = per.tile([128, T, 4, 2, 8], F32)   # t, hc, gh, n8

            v4 = per.tile([128, 4, 2, 8], F32)
            u4 = per.tile([128, 4, 2, 8], F32)
            s4 = per.tile([128, 4, 2, 8], F16)
            half = per.tile([128, 1], F32)
            nc.vector.memset(half, 0.5)

            xh3_all = per.tile([128, T, 392], F32)
            for t in range(8):
                vcur, vprev = lif3_v[t % 2], lif3_v[1 - t % 2]
                if t == 0:
                    nc.vector.tensor_scalar(
                        s3_all[:, 0, :], y3_all[:, 0, :], thr3[:, :], None,
                        op0=OP.is_ge)
                    nc.scalar.activation(vcur, y3_all[:, t, :], AF.Identity,
                                         bias=hc3[:, :], scale=ha3[:, :])
                else:
                    nc.scalar.activation(xh3_all[:, t, :], y3_all[:, t, :],
                                         AF.Identity, bias=hc3[:, :],
                                         scale=ha3[:, :])
                    u3 = l3p.tile([128, 392], F32, name=f"u3_{t}", tag="u3", bufs=2)
                    nc.vector.scalar_tensor_tensor(
                        u3, vprev, 1.0, vprev, op0=OP.is_lt, op1=OP.mult)
                    nc.vector.scalar_tensor_tensor(
                        vcur, u3, 0.5, xh3_all[:, t, :], op0=OP.mult, op1=OP.add)
                    nc.vector.tensor_scalar(
                        s3_all[:, t, :], vcur, 1.0, None, op0=OP.is_ge)

                # fc1 per t-group: accumulate 49 ij into psF[hc][:, gh, ...]
                if t % FC1_TG == FC1_TG - 1 and not NO_FC1:
                    t0 = t - (FC1_TG - 1)
                    s3v = s3_all.rearrange("p tt (n i j) -> p tt n i j", n=8, i=7)
                    for hc in range(4):
                        for gh in range(2):
                            psF = psfp.tile([128, FC1_TG * 8], F32, tag="psf",
                                            bufs=4)
                            for ij in range(49):
                                i, j = ij // 7, ij % 7
                                rhs = s3v[gh * 64:(gh + 1) * 64,
                                          t0:t0 + FC1_TG, :, i, j]
                                nc.tensor.matmul(
                                    psF,
                                    wf4[gh * 64:(gh + 1) * 64, ij, hc, :], rhs,
                                    start=(ij == 0), stop=(ij == 48),
                                    tile_position=(64 * gh, 0))
                            if not NO_LIF4:
                                nc.scalar.activation(
                                    xh4[:, t0:t0 + FC1_TG, hc, gh, :],
                                    psF.rearrange("p (a b) -> p a b", a=FC1_TG),
                                    AF.Identity, bias=vecs[:, 6 + hc:7 + hc],
                                    scale=half[:, :])
                    for tt in ([] if NO_LIF4 else range(t0, t0 + FC1_TG)):
                        if tt == 0:
                            nc.vector.tensor_copy(v4, xh4[:, 0, :, :, :])
                        else:
                            nc.vector.scalar_tensor_tensor(
                                u4, v4, 1.0, v4, op0=OP.is_lt, op1=OP.mult)
                            nc.vector.scalar_tensor_tensor(
                                v4, u4, 0.5, xh4[:, tt, :, :, :],
                                op0=OP.mult, op1=OP.add)
                        nc.vector.tensor_scalar(s4, v4, 1.0, None, op0=OP.is_ge)
                        psO = psop.tile([2, 16], F32, tag="pso", bufs=2)
                        for hc in range(4):
                            nc.tensor.matmul(
                                psO, wor[:, hc, :],
                                s4[:, hc, :, :].rearrange("p g n -> p (g n)"),
                                start=(hc == 0), stop=(hc == 3))
                        if tt == 0:
                            nc.vector.tensor_copy(osum, psO)
                        else:
                            nc.vector.tensor_tensor(osum, osum, psO, op=OP.add)

            if DEBUG:
                nc.sync.dma_start(out=dbg["d_xh4"].ap(), in_=xh4[:, :, :, 0, :])
                nc.sync.dma_start(out=dbg["d_s3"].ap(), in_=s3_all)

            if NO_FC1 or NO_LIF4:
                nc.sync.dma_start(out=out_d.ap(), in_=acc3[0:4, :])
                return
            nc.vector.tensor_scalar(
                out_t, osum, 0.125, vecs[0:2, 10:11], op0=OP.mult, op1=OP.add)

        nc.sync.dma_start(out=out_d.ap(), in_=out_t)


def kernel(**inputs) -> np.ndarray:
    x = np.asarray(inputs["x"], np.float32)
    B = x.shape[0]
    assert B == N_CORES * B_LOC

    if "nc" not in _CACHE:
        _CACHE["nc"] = build_nc()
    nc = _CACHE["nc"]

    shared = _prep_shared(inputs)
    in_maps = []
    for c in range(N_CORES):
        m = dict(shared)
        m.update(_prep_core(x[c * B_LOC:(c + 1) * B_LOC]))
        in_maps.append(m)

    trace = bool(int(os.environ.get("KERNEL_TRACE", "0")))
    res = run_bass_kernel_spmd(nc, in_maps, core_ids=list(range(N_CORES)),
                               trace=trace)
    _CACHE["last_results"] = res
    out = np.concatenate([r["out"].T for r in res.results], axis=0)
    return np.ascontiguousarray(out.astype(np.float32))
